# revision 17
# baseline (speedup 1.0000x reference)
"""Fused causal attention head (QKV proj + causal softmax attention) on 8 trn2 cores.

Sharding: core = 4*b + r (b = batch of 2, r = rank in a 4-core group).
  - Queries: core handles row chunks [512r, 512r+512) and [512(7-r), 512(8-r))
    of its batch (pairing r with 7-r balances causal attention work).
  - K/V: core projects keys [1024r, 1024(r+1)); shards exchanged with four
    pipelined AllGathers (K and V, split by rank-local key half) inside each
    4-core batch group so attention on the first half overlaps the rest.
Attention is computed in a transposed-scores layout (keys on PSUM partitions):
  S^T = K^T-chunks^T-matmul-Q^T, P^T = exp(S^T/32) * causal01mask,
  O = sum_k P^T-slices^T @ V  (PSUM-accumulated), rowsum via ones-matmul,
  O /= rowsum.  No max-subtraction: scores are ~N(0,1) so exp cannot overflow.
Key blocks are processed even-before-odd (kb = 2*rank + half) so each query
group only needs the half-AllGathers that have already completed.
"""

import os
import sys

sys.path.insert(0, "/opt/trn_rl_repo")

import numpy as np
import ml_dtypes

B, S, D = 2, 4096, 1024
NCORES = 8
P = 128
NQ = 1024          # queries per core
QG = 256           # query group (scores matmul free dim)
NQG = NQ // QG     # 4
KBS = (4, 4, 8, 8)  # key blocks (512 keys each) processed per query group
NSTEP = sum(KBS)   # 24
KB = 512           # key block size
BF16 = ml_dtypes.bfloat16

LAST_EXEC_NS = None

WARMUP = int(os.environ.get("KWARMUP", "40"))

_built = {}


def _kb_order(nkb):
    """Even key blocks (rank-half 0) first, then odd — matches AG pipeline."""
    return list(range(0, nkb, 2)) + list(range(1, nkb, 2))


def _build():
    import concourse.bacc as bacc
    import concourse.tile as tile
    import concourse.mybir as mybir
    from concourse.masks import make_identity

    nc = bacc.Bacc("TRN2", target_bir_lowering=False, debug=False,
                   num_devices=NCORES)
    dt = mybir.dt

    xq_t = nc.dram_tensor("xq_t", [D, NQ], dt.bfloat16, kind="ExternalInput").ap()
    xkv_t = nc.dram_tensor("xkv_t", [D, 1024], dt.bfloat16, kind="ExternalInput").ap()
    w = nc.dram_tensor("w", [D, 3 * D], dt.bfloat16, kind="ExternalInput").ap()
    maskt = nc.dram_tensor("maskt", [NSTEP, P, 4, QG], dt.bfloat16,
                           kind="ExternalInput").ap()
    out = nc.dram_tensor("out", [NQ, D], dt.float32, kind="ExternalOutput").ap()

    DC = D // P  # 8 contraction chunks
    RG = [[0, 1, 2, 3], [4, 5, 6, 7]]

    with tile.TileContext(nc, num_cores=NCORES) as tc:
        with (
            tc.tile_pool(name="persist", bufs=1) as persist,
            tc.tile_pool(name="dram", bufs=1, space="DRAM") as dram,
        ):
            qt_sb = persist.tile([P, DC, NQ], dt.bfloat16)
            ones_sb = persist.tile([P, P], dt.bfloat16)
            nc.vector.memset(ones_sb, 1.0)
            ident = persist.tile([P, P], dt.float32)
            make_identity(nc, ident)

            # per key-half AllGather buffers (K^T as [dout, 512], V as [512, dout])
            agin_k = [dram.tile([1024, KB], dt.bfloat16, name=f"agin_k{h}")
                      for h in range(2)]
            agout_k = [dram.tile([4096, KB], dt.bfloat16, name=f"agout_k{h}")
                       for h in range(2)]
            agin_v = [dram.tile([KB, 1024], dt.bfloat16, name=f"agin_v{h}")
                      for h in range(2)]
            agout_v = [dram.tile([4 * KB, 1024], dt.bfloat16,
                                 name=f"agout_v{h}") for h in range(2)]

            # ---- Phase 1: projections + pipelined AllGathers ----
            with (
                tc.tile_pool(name="projbuf", bufs=1) as projbuf,
                tc.tile_pool(name="projtmp", bufs=4) as projtmp,
                tc.tile_pool(name="projps", bufs=4, space="PSUM") as projps,
            ):
                # PE warmup while input DMAs stream
                if WARMUP:
                    wu = projbuf.tile([P, KB], dt.bfloat16)
                    nc.vector.memset(wu, 0.0)
                    wu_ps = projps.tile([P, KB], dt.float32, tag="pps",
                                        name="wu_ps")
                    for i in range(WARMUP):
                        nc.tensor.matmul(wu_ps, lhsT=wu[:, :P], rhs=wu,
                                         start=True, stop=True)

                w_sb = projbuf.tile([P, DC, 3 * D], dt.bfloat16)
                xkv_sb = projbuf.tile([P, DC, 1024], dt.bfloat16)
                xq_sb = projbuf.tile([P, DC, NQ], dt.bfloat16)
                nc.sync.dma_start(xkv_sb, xkv_t.rearrange("(c p) n -> p c n", p=P))
                # W columns in use-order: Wk, Wv, Wq
                w_r = w.rearrange("(c p) n -> p c n", p=P)
                nc.sync.dma_start(w_sb[:, :, D:2 * D], w_r[:, :, D:2 * D])
                nc.sync.dma_start(w_sb[:, :, 2 * D:3 * D], w_r[:, :, 2 * D:3 * D])
                nc.sync.dma_start(w_sb[:, :, 0:D], w_r[:, :, 0:D])
                nc.sync.dma_start(xq_sb, xq_t.rearrange("(c p) n -> p c n", p=P))

                for h in range(2):
                    # K^T half h: [dout, 512 keys]
                    agin_k_r = agin_k[h].rearrange("(m p) k -> m p k", p=P)
                    for m in range(DC):
                        kt_ps = projps.tile([P, KB], dt.float32, tag="pps",
                                            name="kt_ps")
                        for c in range(DC):
                            nc.tensor.matmul(
                                kt_ps,
                                lhsT=w_sb[:, c, D + m * P:D + (m + 1) * P],
                                rhs=xkv_sb[:, c, h * KB:(h + 1) * KB],
                                start=(c == 0), stop=(c == DC - 1),
                            )
                        kt_bf = projtmp.tile([P, KB], dt.bfloat16, tag="pcopy")
                        nc.vector.tensor_copy(kt_bf, kt_ps)
                        nc.sync.dma_start(agin_k_r[m], kt_bf)
                    nc.gpsimd.collective_compute(
                        "AllGather", mybir.AluOpType.bypass,
                        replica_groups=RG,
                        ins=[agin_k[h].opt()], outs=[agout_k[h].opt()],
                    )

                    # V half h: [512 keys, dout]
                    agin_v_r = agin_v[h].rearrange("(m p) d -> m p d", p=P)
                    for m in range(4):
                        for nh in range(2):
                            v_ps = projps.tile([P, KB], dt.float32, tag="pps",
                                               name="v_ps")
                            for c in range(DC):
                                nc.tensor.matmul(
                                    v_ps,
                                    lhsT=xkv_sb[:, c,
                                                h * KB + m * P:
                                                h * KB + (m + 1) * P],
                                    rhs=w_sb[:, c,
                                             2 * D + nh * KB:
                                             2 * D + (nh + 1) * KB],
                                    start=(c == 0), stop=(c == DC - 1),
                                )
                            v_bf = projtmp.tile([P, KB], dt.bfloat16,
                                                tag="pcopy")
                            nc.vector.tensor_copy(v_bf, v_ps)
                            nc.sync.dma_start(
                                agin_v_r[m][:, nh * KB:(nh + 1) * KB], v_bf)
                    nc.gpsimd.collective_compute(
                        "AllGather", mybir.AluOpType.bypass,
                        replica_groups=RG,
                        ins=[agin_v[h].opt()], outs=[agout_v[h].opt()],
                    )

                # Q^T: [dout, q]  (overlaps the AllGathers)
                for m in range(DC):
                    for nh in range(2):
                        q_ps = projps.tile([P, KB], dt.float32, tag="pps",
                                           name="q_ps")
                        for c in range(DC):
                            nc.tensor.matmul(
                                q_ps,
                                lhsT=w_sb[:, c, m * P:(m + 1) * P],
                                rhs=xq_sb[:, c, nh * KB:(nh + 1) * KB],
                                start=(c == 0), stop=(c == DC - 1),
                            )
                        nc.vector.tensor_copy(
                            qt_sb[:, m, nh * KB:(nh + 1) * KB], q_ps)

            # ---- Phase 2: attention ----
            _phase2(nc, tc, mybir, qt_sb, ones_sb, ident,
                    agout_k, agout_v, maskt, out)

    nc.compile()
    return nc


def _phase2(nc, tc, mybir, qt_sb, ones_sb, ident, agout_k, agout_v, maskt,
            out):
    dt = mybir.dt
    DC = D // P
    with (
        tc.tile_pool(name="kv", bufs=1) as kvpool,
        tc.tile_pool(name="mask", bufs=3) as maskpool,
        tc.tile_pool(name="pt", bufs=3) as ptpool,
        tc.tile_pool(name="norm", bufs=2) as normpool,
        tc.tile_pool(name="osb", bufs=2) as osbpool,
        tc.tile_pool(name="ops", bufs=2, space="PSUM") as opspool,
        tc.tile_pool(name="stps", bufs=2, space="PSUM") as stpspool,
        tc.tile_pool(name="sumps", bufs=1, space="PSUM") as sumpspool,
        tc.tile_pool(name="sumtps", bufs=1, space="PSUM") as sumtpspool,
    ):
        # cache all gathered K^T / V blocks in SBUF; kb = 2*rank + half
        kt_blk = {}
        v_blk = {}
        for h in range(2):
            for rr in range(4):
                ktb = kvpool.tile([P, DC, KB], dt.bfloat16,
                                  name=f"ktb{rr}_{h}")
                nc.sync.dma_start(
                    ktb,
                    agout_k[h][1024 * rr:1024 * (rr + 1)]
                    .rearrange("(c p) k -> p c k", p=P))
                kt_blk[2 * rr + h] = ktb
                vb = kvpool.tile([P, 4, 1024], dt.bfloat16,
                                 name=f"vb{rr}_{h}")
                nc.sync.dma_start(
                    vb,
                    agout_v[h][KB * rr:KB * (rr + 1)]
                    .rearrange("(c p) d -> p c d", p=P))
                v_blk[2 * rr + h] = vb

        step = 0
        for qg in range(NQG):
            qoff = qg * QG
            nkb = KBS[qg]
            order = _kb_order(nkb)
            o_ps = [opspool.tile([P, D], dt.float32, tag="o_ps",
                                 name=f"o_qg{qg}_{qs}") for qs in range(2)]
            sum_ps = sumpspool.tile([P, QG], dt.float32, tag="sum_ps")
            for oi, kb in enumerate(order):
                ktb = kt_blk[kb]
                vb = v_blk[kb]
                mask_sb = maskpool.tile([P, 4, QG], dt.bfloat16, tag="mask")
                nc.sync.dma_start(mask_sb, maskt[step])
                first = oi == 0
                last = oi == nkb - 1
                for kt in range(4):
                    st_ps = stpspool.tile([P, QG], dt.float32, tag="st")
                    for c in range(DC):
                        nc.tensor.matmul(
                            st_ps,
                            lhsT=ktb[:, c, kt * P:(kt + 1) * P],
                            rhs=qt_sb[:, c, qoff:qoff + QG],
                            start=(c == 0), stop=(c == DC - 1),
                        )
                    pt_sb = ptpool.tile([P, QG], dt.bfloat16, tag="pt")
                    nc.scalar.activation(
                        out=pt_sb, in_=st_ps,
                        func=mybir.ActivationFunctionType.Exp,
                        scale=float(1.0 / np.sqrt(D)),
                    )
                    nc.vector.tensor_mul(pt_sb, pt_sb, mask_sb[:, kt, :])
                    mm_start = first and kt == 0
                    mm_stop = last and kt == 3
                    for qs in range(2):
                        for dn in range(2):
                            nc.tensor.matmul(
                                o_ps[qs][:, dn * KB:(dn + 1) * KB],
                                lhsT=pt_sb[:, qs * P:(qs + 1) * P],
                                rhs=vb[:, kt, dn * KB:(dn + 1) * KB],
                                start=mm_start, stop=mm_stop,
                            )
                    nc.tensor.matmul(
                        sum_ps, lhsT=ones_sb, rhs=pt_sb,
                        start=mm_start, stop=mm_stop,
                    )
                step += 1

            # normalize: O /= rowsum
            sum_sb = normpool.tile([P, QG], dt.float32, tag="sum_sb")
            nc.vector.tensor_copy(sum_sb, sum_ps)
            for qs in range(2):
                o_sb = osbpool.tile([P, D], dt.float32, tag="o_sb")
                sumt_ps = sumtpspool.tile([P, P], dt.float32, tag="sumt")
                nc.tensor.transpose(
                    sumt_ps, sum_sb[:, qs * P:(qs + 1) * P], ident)
                recip = normpool.tile([P, 1], dt.float32, tag="recip")
                nc.vector.reciprocal(recip, sumt_ps[:, 0:1])
                nc.vector.tensor_scalar_mul(o_sb, o_ps[qs], recip)
                nc.sync.dma_start(
                    out[qoff + qs * P:qoff + (qs + 1) * P, :], o_sb)


def _get_nc():
    if "nc" not in _built:
        _built["nc"] = _build()
    return _built["nc"]


def _host_inputs(x, W):
    """Build the 8 per-core input maps from the full inputs."""
    x = np.asarray(x)
    W = np.asarray(W)
    w_bf = W.astype(BF16)

    in_maps = []
    for core in range(NCORES):
        b, r = divmod(core, 4)
        rows_a = slice(512 * r, 512 * r + 512)
        rows_b = slice(512 * (7 - r), 512 * (7 - r) + 512)
        xq = np.concatenate([x[b, rows_a], x[b, rows_b]], axis=0)  # [1024, D]
        xkv = x[b, 1024 * r:1024 * (r + 1)]                        # [1024, D]
        in_maps.append({
            "xq_t": np.ascontiguousarray(xq.T).astype(BF16),
            "xkv_t": np.ascontiguousarray(xkv.T).astype(BF16),
            "w": w_bf,
            "maskt": _masks_for_rank(r),
        })
    return in_maps


_mask_cache = {}


def _masks_for_rank(r):
    if r in _mask_cache:
        return _mask_cache[r]
    # global query position of per-core query index j
    qpos = np.empty(NQ, dtype=np.int64)
    qpos[:512] = 512 * r + np.arange(512)
    qpos[512:] = 512 * (7 - r) + np.arange(512)
    m = np.zeros((NSTEP, P, 4, QG), dtype=BF16)
    step = 0
    for qg in range(NQG):
        qp = qpos[qg * QG:(qg + 1) * QG]  # [QG]
        for kb in _kb_order(KBS[qg]):
            for kt in range(4):
                kpos = KB * kb + kt * P + np.arange(P)  # [P]
                m[step, :, kt, :] = (kpos[:, None] <= qp[None, :]).astype(BF16)
            step += 1
    assert step == NSTEP
    _mask_cache[r] = m
    return m


def _gather(results):
    out = np.empty((B, S, D), dtype=np.float32)
    for core in range(NCORES):
        b, r = divmod(core, 4)
        co = results[core]["out"]
        out[b, 512 * r:512 * r + 512] = co[:512]
        out[b, 512 * (7 - r):512 * (7 - r) + 512] = co[512:]
    return out


def kernel(x, W):
    global LAST_EXEC_NS
    from concourse import bass_utils

    nc = _get_nc()
    in_maps = _host_inputs(x, W)
    trace = os.environ.get("BASS_KERNEL_TRACE", "0") == "1"
    if trace:
        try:
            import antenv.axon_hooks as ah
            ah.install_default_hook()
        except Exception:
            pass
    res = bass_utils.run_bass_kernel_spmd(
        nc, in_maps, core_ids=list(range(NCORES)), trace=trace,
        tmpdir=os.environ.get("BASS_KERNEL_TRACE_DIR") or None,
    )
    LAST_EXEC_NS = res.exec_time_ns
    return _gather(res.results)


# revision 18
# speedup vs baseline: 1.0724x; 1.0724x over previous
"""Fused causal attention head (QKV proj + causal softmax attention) on 8 trn2 cores.

Sharding: core = 4*b + r (b = batch of 2, r = rank in a 4-core group).
  - Queries: core handles row chunks [512r, 512r+512) and [512(7-r), 512(8-r))
    of its batch (pairing r with 7-r balances causal attention work).
  - K/V: core projects keys [1024r, 1024(r+1)); shards are exchanged with 8
    pipelined AllGathers (K and V per rank-local key *quarter*) inside each
    4-core batch group. Attention consumes quarters as they arrive
    (quarter-major loop), hiding the ~65 GB/s interconnect behind compute.
Attention is computed in a transposed-scores layout (keys on PSUM partitions):
  S^T = K^T-chunk^T-matmul-Q^T, P^T = exp(S^T/32) * causal01mask,
  partial O/rowsum accumulate in PSUM per (quarter, query-group), then fold
  into SBUF fp32 accumulators; final O /= rowsum.
  No max-subtraction: scores are ~N(0,1) so exp cannot overflow fp32.
"""

import os
import sys

sys.path.insert(0, "/opt/trn_rl_repo")

import numpy as np
import ml_dtypes

B, S, D = 2, 4096, 1024
NCORES = 8
P = 128
NQ = 1024          # queries per core
QG = 256           # query group (scores matmul free dim)
NQG = NQ // QG     # 4
KB = 512
QK = 256           # quarter-of-rank key block
BF16 = ml_dtypes.bfloat16

# ranks visible per query group (qg 0/1 = early chunk, keys < 2048)
RRS = ((0, 1), (0, 1), (0, 1, 2, 3), (0, 1, 2, 3))
NSTEP = 4 * sum(len(r) for r in RRS)  # 48 mask tiles

LAST_EXEC_NS = None
WARMUP = int(os.environ.get("KWARMUP", "40"))

_built = {}


def _build():
    import concourse.bacc as bacc
    import concourse.tile as tile
    import concourse.mybir as mybir
    from concourse.masks import make_identity

    nc = bacc.Bacc("TRN2", target_bir_lowering=False, debug=False,
                   num_devices=NCORES)
    dt = mybir.dt

    xq_t = nc.dram_tensor("xq_t", [D, NQ], dt.bfloat16, kind="ExternalInput").ap()
    xkv_t = nc.dram_tensor("xkv_t", [D, 1024], dt.bfloat16, kind="ExternalInput").ap()
    w = nc.dram_tensor("w", [D, 3 * D], dt.bfloat16, kind="ExternalInput").ap()
    maskt = nc.dram_tensor("maskt", [NSTEP, P, 2, QG], dt.bfloat16,
                           kind="ExternalInput").ap()
    out = nc.dram_tensor("out", [NQ, D], dt.float32, kind="ExternalOutput").ap()

    DC = D // P  # 8 contraction chunks
    RG = [[0, 1, 2, 3], [4, 5, 6, 7]]

    with tile.TileContext(nc, num_cores=NCORES) as tc:
        with (
            tc.tile_pool(name="persist", bufs=1) as persist,
            tc.tile_pool(name="dram", bufs=1, space="DRAM") as dram,
        ):
            qt_sb = persist.tile([P, DC, NQ], dt.bfloat16)
            ones_sb = persist.tile([P, P], dt.bfloat16)
            nc.vector.memset(ones_sb, 1.0)
            ident = persist.tile([P, P], dt.float32)
            make_identity(nc, ident)

            agin_k = [dram.tile([1024, QK], dt.bfloat16, name=f"agin_k{q}")
                      for q in range(4)]
            agout_k = [dram.tile([4096, QK], dt.bfloat16, name=f"agout_k{q}")
                       for q in range(4)]
            agin_v = [dram.tile([QK, 1024], dt.bfloat16, name=f"agin_v{q}")
                      for q in range(4)]
            agout_v = [dram.tile([4 * QK, 1024], dt.bfloat16,
                                 name=f"agout_v{q}") for q in range(4)]

            # ---- Phase 1: projections + 8 pipelined AllGathers ----
            with (
                tc.tile_pool(name="projbuf", bufs=1) as projbuf,
                tc.tile_pool(name="projtmp", bufs=4) as projtmp,
                tc.tile_pool(name="projps", bufs=4, space="PSUM") as projps,
            ):
                # absorb the one-time collective barrier with a tiny AG
                dum_sb = projbuf.tile([1, 16], dt.bfloat16)
                nc.vector.memset(dum_sb, 0.0)
                dum_in = dram.tile([1, 16], dt.bfloat16)
                dum_out = dram.tile([4, 16], dt.bfloat16)
                nc.sync.dma_start(dum_in, dum_sb)
                nc.gpsimd.collective_compute(
                    "AllGather", mybir.AluOpType.bypass, replica_groups=RG,
                    ins=[dum_in.opt()], outs=[dum_out.opt()])

                # PE warmup while input DMAs stream
                if WARMUP:
                    wu = projbuf.tile([P, KB], dt.bfloat16)
                    nc.vector.memset(wu, 0.0)
                    wu_ps = projps.tile([P, KB], dt.float32, tag="pps",
                                        name="wu_ps")
                    for i in range(WARMUP):
                        nc.tensor.matmul(wu_ps, lhsT=wu[:, :P], rhs=wu,
                                         start=True, stop=True)

                w_sb = projbuf.tile([P, DC, 3 * D], dt.bfloat16)
                xkv_sb = projbuf.tile([P, DC, 1024], dt.bfloat16)
                xq_sb = projbuf.tile([P, DC, NQ], dt.bfloat16)
                nc.sync.dma_start(xkv_sb, xkv_t.rearrange("(c p) n -> p c n", p=P))
                w_r = w.rearrange("(c p) n -> p c n", p=P)
                nc.sync.dma_start(w_sb[:, :, D:2 * D], w_r[:, :, D:2 * D])
                nc.sync.dma_start(w_sb[:, :, 2 * D:3 * D], w_r[:, :, 2 * D:3 * D])
                nc.sync.dma_start(w_sb[:, :, 0:D], w_r[:, :, 0:D])
                nc.sync.dma_start(xq_sb, xq_t.rearrange("(c p) n -> p c n", p=P))

                for q in range(4):
                    # K^T quarter q: [dout, QK keys]
                    agin_k_r = agin_k[q].rearrange("(m p) k -> m p k", p=P)
                    for m in range(DC):
                        kt_ps = projps.tile([P, QK], dt.float32, tag="ppsk",
                                            name="kt_ps")
                        for c in range(DC):
                            nc.tensor.matmul(
                                kt_ps,
                                lhsT=w_sb[:, c, D + m * P:D + (m + 1) * P],
                                rhs=xkv_sb[:, c, q * QK:(q + 1) * QK],
                                start=(c == 0), stop=(c == DC - 1),
                            )
                        kt_bf = projtmp.tile([P, QK], dt.bfloat16, tag="pck")
                        nc.vector.tensor_copy(kt_bf, kt_ps)
                        nc.sync.dma_start(agin_k_r[m], kt_bf)
                    nc.gpsimd.collective_compute(
                        "AllGather", mybir.AluOpType.bypass, replica_groups=RG,
                        ins=[agin_k[q].opt()], outs=[agout_k[q].opt()])

                    # V quarter q: [QK keys, dout]
                    agin_v_r = agin_v[q].rearrange("(m p) d -> m p d", p=P)
                    for m in range(2):
                        for nh in range(2):
                            v_ps = projps.tile([P, KB], dt.float32, tag="pps",
                                               name="v_ps")
                            for c in range(DC):
                                nc.tensor.matmul(
                                    v_ps,
                                    lhsT=xkv_sb[:, c,
                                                q * QK + m * P:
                                                q * QK + (m + 1) * P],
                                    rhs=w_sb[:, c,
                                             2 * D + nh * KB:
                                             2 * D + (nh + 1) * KB],
                                    start=(c == 0), stop=(c == DC - 1),
                                )
                            v_bf = projtmp.tile([P, KB], dt.bfloat16,
                                                tag="pcopy")
                            nc.vector.tensor_copy(v_bf, v_ps)
                            nc.sync.dma_start(
                                agin_v_r[m][:, nh * KB:(nh + 1) * KB], v_bf)
                    nc.gpsimd.collective_compute(
                        "AllGather", mybir.AluOpType.bypass, replica_groups=RG,
                        ins=[agin_v[q].opt()], outs=[agout_v[q].opt()])

                # Q^T: [dout, q]  (overlaps the AllGathers)
                for m in range(DC):
                    for nh in range(2):
                        q_ps = projps.tile([P, KB], dt.float32, tag="pps",
                                           name="q_ps")
                        for c in range(DC):
                            nc.tensor.matmul(
                                q_ps,
                                lhsT=w_sb[:, c, m * P:(m + 1) * P],
                                rhs=xq_sb[:, c, nh * KB:(nh + 1) * KB],
                                start=(c == 0), stop=(c == DC - 1),
                            )
                        nc.vector.tensor_copy(
                            qt_sb[:, m, nh * KB:(nh + 1) * KB], q_ps)

            # ---- Phase 2: attention, quarter-major ----
            _phase2(nc, tc, mybir, qt_sb, ones_sb, ident,
                    agout_k, agout_v, maskt, out)

    nc.compile()
    return nc


def _phase2(nc, tc, mybir, qt_sb, ones_sb, ident, agout_k, agout_v, maskt,
            out):
    dt = mybir.dt
    DC = D // P
    with (
        tc.tile_pool(name="acc", bufs=1) as accpool,
        tc.tile_pool(name="kvq", bufs=2) as kvqpool,
        tc.tile_pool(name="mask", bufs=4) as maskpool,
        tc.tile_pool(name="pt", bufs=3) as ptpool,
        tc.tile_pool(name="norm", bufs=2) as normpool,
        tc.tile_pool(name="osb", bufs=2) as osbpool,
        tc.tile_pool(name="ops", bufs=2, space="PSUM") as opspool,
        tc.tile_pool(name="stps", bufs=2, space="PSUM") as stpspool,
        tc.tile_pool(name="sumps", bufs=1, space="PSUM") as sumpspool,
        tc.tile_pool(name="sumtps", bufs=1, space="PSUM") as sumtpspool,
    ):
        o_acc = [[accpool.tile([P, D], dt.float32, name=f"oacc{qg}_{qs}")
                  for qs in range(2)] for qg in range(NQG)]
        sum_acc = [accpool.tile([P, QG], dt.float32, name=f"sacc{qg}")
                   for qg in range(NQG)]

        step = 0
        for q in range(4):
            # stream in this quarter's K^T / V for all 4 ranks
            ktq = []
            vq = []
            for rr in range(4):
                kt_t = kvqpool.tile([P, DC, QK], dt.bfloat16, tag=f"ktq{rr}",
                                    name=f"ktq{q}_{rr}")
                nc.sync.dma_start(
                    kt_t,
                    agout_k[q][1024 * rr:1024 * (rr + 1)]
                    .rearrange("(c p) k -> p c k", p=P))
                ktq.append(kt_t)
                v_t = kvqpool.tile([P, 2, 1024], dt.bfloat16, tag=f"vq{rr}",
                                   name=f"vq{q}_{rr}")
                nc.sync.dma_start(
                    v_t,
                    agout_v[q][QK * rr:QK * (rr + 1)]
                    .rearrange("(c p) d -> p c d", p=P))
                vq.append(v_t)

            for qg in range(NQG):
                qoff = qg * QG
                rrs = RRS[qg]
                o_ps = [opspool.tile([P, D], dt.float32, tag="o_ps",
                                     name=f"o_{q}_{qg}_{qs}")
                        for qs in range(2)]
                sum_ps = sumpspool.tile([P, QG], dt.float32, tag="sum_ps")
                for rr in rrs:
                    mask_sb = maskpool.tile([P, 2, QG], dt.bfloat16,
                                            tag="mask")
                    nc.sync.dma_start(mask_sb, maskt[step])
                    for kt in range(2):
                        st_ps = stpspool.tile([P, QG], dt.float32, tag="st")
                        for c in range(DC):
                            nc.tensor.matmul(
                                st_ps,
                                lhsT=ktq[rr][:, c, kt * P:(kt + 1) * P],
                                rhs=qt_sb[:, c, qoff:qoff + QG],
                                start=(c == 0), stop=(c == DC - 1),
                            )
                        pt_sb = ptpool.tile([P, QG], dt.bfloat16, tag="pt")
                        nc.scalar.activation(
                            out=pt_sb, in_=st_ps,
                            func=mybir.ActivationFunctionType.Exp,
                            scale=float(1.0 / np.sqrt(D)),
                        )
                        nc.vector.tensor_mul(pt_sb, pt_sb, mask_sb[:, kt, :])
                        mm_start = rr == rrs[0] and kt == 0
                        mm_stop = rr == rrs[-1] and kt == 1
                        for qs in range(2):
                            for dn in range(2):
                                nc.tensor.matmul(
                                    o_ps[qs][:, dn * KB:(dn + 1) * KB],
                                    lhsT=pt_sb[:, qs * P:(qs + 1) * P],
                                    rhs=vq[rr][:, kt, dn * KB:(dn + 1) * KB],
                                    start=mm_start, stop=mm_stop,
                                )
                        nc.tensor.matmul(
                            sum_ps, lhsT=ones_sb, rhs=pt_sb,
                            start=mm_start, stop=mm_stop,
                        )
                    step += 1

                # fold partials into SBUF accumulators
                for qs in range(2):
                    if q == 0:
                        nc.vector.tensor_copy(o_acc[qg][qs], o_ps[qs])
                    else:
                        nc.vector.tensor_add(o_acc[qg][qs], o_acc[qg][qs],
                                             o_ps[qs])
                if q == 0:
                    nc.vector.tensor_copy(sum_acc[qg], sum_ps)
                else:
                    nc.vector.tensor_add(sum_acc[qg], sum_acc[qg], sum_ps)

        assert step == NSTEP

        # ---- normalize: O /= rowsum ----
        for qg in range(NQG):
            qoff = qg * QG
            for qs in range(2):
                o_sb = osbpool.tile([P, D], dt.float32, tag="o_sb")
                sumt_ps = sumtpspool.tile([P, P], dt.float32, tag="sumt")
                nc.tensor.transpose(
                    sumt_ps, sum_acc[qg][:, qs * P:(qs + 1) * P], ident)
                recip = normpool.tile([P, 1], dt.float32, tag="recip")
                nc.vector.reciprocal(recip, sumt_ps[:, 0:1])
                nc.vector.tensor_scalar_mul(o_sb, o_acc[qg][qs], recip)
                nc.sync.dma_start(
                    out[qoff + qs * P:qoff + (qs + 1) * P, :], o_sb)


def _get_nc():
    if "nc" not in _built:
        _built["nc"] = _build()
    return _built["nc"]


def _host_inputs(x, W):
    """Build the 8 per-core input maps from the full inputs."""
    x = np.asarray(x)
    W = np.asarray(W)
    w_bf = W.astype(BF16)

    in_maps = []
    for core in range(NCORES):
        b, r = divmod(core, 4)
        rows_a = slice(512 * r, 512 * r + 512)
        rows_b = slice(512 * (7 - r), 512 * (7 - r) + 512)
        xq = np.concatenate([x[b, rows_a], x[b, rows_b]], axis=0)  # [1024, D]
        xkv = x[b, 1024 * r:1024 * (r + 1)]                        # [1024, D]
        in_maps.append({
            "xq_t": np.ascontiguousarray(xq.T).astype(BF16),
            "xkv_t": np.ascontiguousarray(xkv.T).astype(BF16),
            "w": w_bf,
            "maskt": _masks_for_rank(r),
        })
    return in_maps


_mask_cache = {}


def _masks_for_rank(r):
    if r in _mask_cache:
        return _mask_cache[r]
    qpos = np.empty(NQ, dtype=np.int64)
    qpos[:512] = 512 * r + np.arange(512)
    qpos[512:] = 512 * (7 - r) + np.arange(512)
    m = np.zeros((NSTEP, P, 2, QG), dtype=BF16)
    step = 0
    for q in range(4):
        for qg in range(NQG):
            qp = qpos[qg * QG:(qg + 1) * QG]
            for rr in RRS[qg]:
                for kt in range(2):
                    kpos = 1024 * rr + QK * q + kt * P + np.arange(P)
                    m[step, :, kt, :] = (
                        kpos[:, None] <= qp[None, :]).astype(BF16)
                step += 1
    assert step == NSTEP
    _mask_cache[r] = m
    return m


def _gather(results):
    out = np.empty((B, S, D), dtype=np.float32)
    for core in range(NCORES):
        b, r = divmod(core, 4)
        co = results[core]["out"]
        out[b, 512 * r:512 * r + 512] = co[:512]
        out[b, 512 * (7 - r):512 * (7 - r) + 512] = co[512:]
    return out


def kernel(x, W):
    global LAST_EXEC_NS
    from concourse import bass_utils

    nc = _get_nc()
    in_maps = _host_inputs(x, W)
    trace = os.environ.get("BASS_KERNEL_TRACE", "0") == "1"
    if trace:
        try:
            import antenv.axon_hooks as ah
            ah.install_default_hook()
        except Exception:
            pass
    res = bass_utils.run_bass_kernel_spmd(
        nc, in_maps, core_ids=list(range(NCORES)), trace=trace,
        tmpdir=os.environ.get("BASS_KERNEL_TRACE_DIR") or None,
    )
    LAST_EXEC_NS = res.exec_time_ns
    return _gather(res.results)


# revision 25
# speedup vs baseline: 1.1108x; 1.0358x over previous
"""Fused causal attention head (QKV proj + causal softmax attention) on 8 trn2 cores.

Sharding: core = 4*b + r (b = batch of 2, r = rank in a 4-core group).
  - Queries: core handles row chunks [512r, 512r+512) and [512(7-r), 512(8-r))
    of its batch (pairing r with 7-r balances causal attention work).
  - K/V: core projects keys [1024r, 1024(r+1)); shards are exchanged with 8
    pipelined AllGathers (K and V per rank-local key *quarter*) inside each
    4-core batch group. Attention consumes quarters as they arrive
    (quarter-major loop), hiding the ~65 GB/s interconnect behind compute.
Attention is computed in a transposed-scores layout (keys on PSUM partitions):
  S^T = K^T-chunk^T-matmul-Q^T, P^T = exp(S^T/32) * causal01mask,
  partial O/rowsum accumulate in PSUM per (quarter, query-group), then fold
  into SBUF fp32 accumulators; final O /= rowsum.
  No max-subtraction: scores are ~N(0,1) so exp cannot overflow fp32.
"""

import os
import sys

sys.path.insert(0, "/opt/trn_rl_repo")

import numpy as np
import ml_dtypes

B, S, D = 2, 4096, 1024
NCORES = 8
P = 128
NQ = 1024          # queries per core
QG = 256           # query group (scores matmul free dim)
NQG = NQ // QG     # 4
KB = 512
QK = 256           # quarter-of-rank key block
BF16 = ml_dtypes.bfloat16

# ranks visible per query group (qg 0/1 = early chunk, keys < 2048)
RRS = ((0, 1), (0, 1), (0, 1, 2, 3), (0, 1, 2, 3))
NSTEP = 4 * sum(len(r) for r in RRS)  # 48 mask tiles

LAST_EXEC_NS = None
WARMUP = int(os.environ.get("KWARMUP", "12"))

_built = {}


def _build():
    import concourse.bacc as bacc
    import concourse.tile as tile
    import concourse.mybir as mybir
    from concourse.masks import make_identity

    nc = bacc.Bacc("TRN2", target_bir_lowering=False, debug=False,
                   num_devices=NCORES)
    dt = mybir.dt

    xq_t = nc.dram_tensor("xq_t", [D, NQ], dt.bfloat16, kind="ExternalInput").ap()
    xkv_t = nc.dram_tensor("xkv_t", [D, 1024], dt.bfloat16, kind="ExternalInput").ap()
    w = nc.dram_tensor("w", [D, 3 * D], dt.bfloat16, kind="ExternalInput").ap()
    maskt = nc.dram_tensor("maskt", [NSTEP, P, 2, QG], dt.bfloat16,
                           kind="ExternalInput").ap()
    out = nc.dram_tensor("out", [NQ, D], dt.float32, kind="ExternalOutput").ap()

    DC = D // P  # 8 contraction chunks
    RG = [[0, 1, 2, 3], [4, 5, 6, 7]]

    with tile.TileContext(nc, num_cores=NCORES) as tc:
        with (
            tc.tile_pool(name="persist", bufs=1) as persist,
            tc.tile_pool(name="dram", bufs=1, space="DRAM") as dram,
        ):
            qt_sb = persist.tile([P, DC, NQ], dt.bfloat16)
            ones_sb = persist.tile([P, P], dt.bfloat16)
            nc.vector.memset(ones_sb, 1.0)
            ident = persist.tile([P, P], dt.float32)
            make_identity(nc, ident)

            agin_k = [dram.tile([1024, QK], dt.bfloat16, name=f"agin_k{q}")
                      for q in range(4)]
            agout_k = [dram.tile([4096, QK], dt.bfloat16, name=f"agout_k{q}")
                       for q in range(4)]
            agin_v = [dram.tile([QK, 1024], dt.bfloat16, name=f"agin_v{q}")
                      for q in range(4)]
            agout_v = [dram.tile([4 * QK, 1024], dt.bfloat16,
                                 name=f"agout_v{q}") for q in range(4)]

            # ---- Phase 1: projections + 8 pipelined AllGathers ----
            with (
                tc.tile_pool(name="projbuf", bufs=1) as projbuf,
                tc.tile_pool(name="projtmp", bufs=4) as projtmp,
                tc.tile_pool(name="projps", bufs=4, space="PSUM") as projps,
            ):
                # PE warmup while input DMAs stream
                if WARMUP:
                    wu = projbuf.tile([P, KB], dt.bfloat16)
                    nc.vector.memset(wu, 0.0)
                    wu_ps = projps.tile([P, KB], dt.float32, tag="pps",
                                        name="wu_ps")
                    for i in range(WARMUP):
                        nc.tensor.matmul(wu_ps, lhsT=wu[:, :P], rhs=wu,
                                         start=True, stop=True)

                w_sb = projbuf.tile([P, DC, 3 * D], dt.bfloat16)
                xkv_sb = projbuf.tile([P, DC, 1024], dt.bfloat16)
                xq_sb = projbuf.tile([P, DC, NQ], dt.bfloat16)
                nc.sync.dma_start(xkv_sb, xkv_t.rearrange("(c p) n -> p c n", p=P))
                w_r = w.rearrange("(c p) n -> p c n", p=P)
                nc.sync.dma_start(w_sb[:, :, D:2 * D], w_r[:, :, D:2 * D])
                nc.sync.dma_start(w_sb[:, :, 2 * D:3 * D], w_r[:, :, 2 * D:3 * D])
                nc.sync.dma_start(w_sb[:, :, 0:D], w_r[:, :, 0:D])
                nc.sync.dma_start(xq_sb, xq_t.rearrange("(c p) n -> p c n", p=P))

                for q in range(4):
                    # K^T quarter q: [dout, QK keys]
                    agin_k_r = agin_k[q].rearrange("(m p) k -> m p k", p=P)
                    for m in range(DC):
                        kt_ps = projps.tile([P, QK], dt.float32, tag="ppsk",
                                            name="kt_ps")
                        for c in range(DC):
                            nc.tensor.matmul(
                                kt_ps,
                                lhsT=w_sb[:, c, D + m * P:D + (m + 1) * P],
                                rhs=xkv_sb[:, c, q * QK:(q + 1) * QK],
                                start=(c == 0), stop=(c == DC - 1),
                            )
                        kt_bf = projtmp.tile([P, QK], dt.bfloat16, tag="pck")
                        nc.vector.tensor_copy(kt_bf, kt_ps)
                        nc.sync.dma_start(agin_k_r[m], kt_bf)
                    nc.gpsimd.collective_compute(
                        "AllGather", mybir.AluOpType.bypass, replica_groups=RG,
                        ins=[agin_k[q].opt()], outs=[agout_k[q].opt()])

                    # V quarter q: [QK keys, dout]
                    agin_v_r = agin_v[q].rearrange("(m p) d -> m p d", p=P)
                    for m in range(2):
                        for nh in range(2):
                            v_ps = projps.tile([P, KB], dt.float32, tag="pps",
                                               name="v_ps")
                            for c in range(DC):
                                nc.tensor.matmul(
                                    v_ps,
                                    lhsT=xkv_sb[:, c,
                                                q * QK + m * P:
                                                q * QK + (m + 1) * P],
                                    rhs=w_sb[:, c,
                                             2 * D + nh * KB:
                                             2 * D + (nh + 1) * KB],
                                    start=(c == 0), stop=(c == DC - 1),
                                )
                            v_bf = projtmp.tile([P, KB], dt.bfloat16,
                                                tag="pcopy")
                            nc.vector.tensor_copy(v_bf, v_ps)
                            nc.sync.dma_start(
                                agin_v_r[m][:, nh * KB:(nh + 1) * KB], v_bf)
                    nc.gpsimd.collective_compute(
                        "AllGather", mybir.AluOpType.bypass, replica_groups=RG,
                        ins=[agin_v[q].opt()], outs=[agout_v[q].opt()])

                # Q^T: [dout, q]  (overlaps the AllGathers)
                for m in range(DC):
                    for nh in range(2):
                        q_ps = projps.tile([P, KB], dt.float32, tag="pps",
                                           name="q_ps")
                        for c in range(DC):
                            nc.tensor.matmul(
                                q_ps,
                                lhsT=w_sb[:, c, m * P:(m + 1) * P],
                                rhs=xq_sb[:, c, nh * KB:(nh + 1) * KB],
                                start=(c == 0), stop=(c == DC - 1),
                            )
                        nc.vector.tensor_copy(
                            qt_sb[:, m, nh * KB:(nh + 1) * KB], q_ps)

            # ---- Phase 2: attention, quarter-major ----
            _phase2(nc, tc, mybir, qt_sb, ones_sb, ident,
                    agout_k, agout_v, maskt, out)

    nc.compile()
    return nc


def _phase2(nc, tc, mybir, qt_sb, ones_sb, ident, agout_k, agout_v, maskt,
            out):
    dt = mybir.dt
    DC = D // P
    with (
        tc.tile_pool(name="acc", bufs=1) as accpool,
        tc.tile_pool(name="kvq", bufs=2) as kvqpool,
        tc.tile_pool(name="mask", bufs=4) as maskpool,
        tc.tile_pool(name="pt", bufs=3) as ptpool,
        tc.tile_pool(name="norm", bufs=2) as normpool,
        tc.tile_pool(name="osb", bufs=2) as osbpool,
        tc.tile_pool(name="ops", bufs=5, space="PSUM") as opspool,
        tc.tile_pool(name="stps", bufs=2, space="PSUM") as stpspool,
        tc.tile_pool(name="sumps", bufs=1, space="PSUM") as sumpspool,
    ):
        o_acc = [[accpool.tile([P, D], dt.float32, name=f"oacc{qg}_{qs}")
                  for qs in range(2)] for qg in range(NQG)]
        sum_acc = [accpool.tile([P, QG], dt.float32, name=f"sacc{qg}")
                   for qg in range(NQG)]

        step = 0
        for q in range(4):
            # stream in this quarter's K^T / V for all 4 ranks
            ktq = []
            vq = []
            for rr in range(4):
                kt_t = kvqpool.tile([P, DC, QK], dt.bfloat16, tag=f"ktq{rr}",
                                    name=f"ktq{q}_{rr}")
                nc.sync.dma_start(
                    kt_t,
                    agout_k[q][1024 * rr:1024 * (rr + 1)]
                    .rearrange("(c p) k -> p c k", p=P))
                ktq.append(kt_t)
                v_t = kvqpool.tile([P, 2, 1024], dt.bfloat16, tag=f"vq{rr}",
                                   name=f"vq{q}_{rr}")
                nc.sync.dma_start(
                    v_t,
                    agout_v[q][QK * rr:QK * (rr + 1)]
                    .rearrange("(c p) d -> p c d", p=P))
                vq.append(v_t)

            for qg in range(NQG):
                qoff = qg * QG
                rrs = RRS[qg]
                # four 1-bank partial-O tiles (qs, dn) with 5 slots so the
                # next (quarter, qg) can start accumulating while folds drain
                o_ps = [opspool.tile([P, KB], dt.float32, tag="opart", bufs=5,
                                     name=f"o_{q}_{qg}_{i}")
                        for i in range(4)]
                sum_ps = sumpspool.tile([P, QG], dt.float32, tag="sum_ps")
                for rr in rrs:
                    mask_sb = maskpool.tile([P, 2, QG], dt.bfloat16,
                                            tag="mask")
                    nc.sync.dma_start(mask_sb, maskt[step])
                    for kt in range(2):
                        st_ps = stpspool.tile([P, QG], dt.float32, tag="st")
                        for c in range(DC):
                            nc.tensor.matmul(
                                st_ps,
                                lhsT=ktq[rr][:, c, kt * P:(kt + 1) * P],
                                rhs=qt_sb[:, c, qoff:qoff + QG],
                                start=(c == 0), stop=(c == DC - 1),
                            )
                        pt_sb = ptpool.tile([P, QG], dt.bfloat16, tag="pt")
                        nc.scalar.activation(
                            out=pt_sb, in_=st_ps,
                            func=mybir.ActivationFunctionType.Exp,
                            scale=float(1.0 / np.sqrt(D)),
                        )
                        nc.vector.tensor_mul(pt_sb, pt_sb, mask_sb[:, kt, :])
                        mm_start = rr == rrs[0] and kt == 0
                        mm_stop = rr == rrs[-1] and kt == 1
                        for qs in range(2):
                            for dn in range(2):
                                nc.tensor.matmul(
                                    o_ps[qs * 2 + dn],
                                    lhsT=pt_sb[:, qs * P:(qs + 1) * P],
                                    rhs=vq[rr][:, kt, dn * KB:(dn + 1) * KB],
                                    start=mm_start, stop=mm_stop,
                                )
                        nc.tensor.matmul(
                            sum_ps, lhsT=ones_sb, rhs=pt_sb,
                            start=mm_start, stop=mm_stop,
                        )
                    step += 1

                # fold partials into SBUF accumulators
                for qs in range(2):
                    for dn in range(2):
                        dst = o_acc[qg][qs][:, dn * KB:(dn + 1) * KB]
                        if q == 0:
                            nc.vector.tensor_copy(dst, o_ps[qs * 2 + dn])
                        else:
                            nc.vector.tensor_add(dst, dst, o_ps[qs * 2 + dn])
                if q == 0:
                    nc.vector.tensor_copy(sum_acc[qg], sum_ps)
                else:
                    nc.vector.tensor_add(sum_acc[qg], sum_acc[qg], sum_ps)

        assert step == NSTEP

        # ---- normalize: O /= rowsum ----
        for qg in range(NQG):
            qoff = qg * QG
            for qs in range(2):
                o_sb = osbpool.tile([P, D], dt.float32, tag="o_sb")
                sumt_ps = stpspool.tile([P, P], dt.float32, tag="st")
                nc.tensor.transpose(
                    sumt_ps, sum_acc[qg][:, qs * P:(qs + 1) * P], ident)
                recip = normpool.tile([P, 1], dt.float32, tag="recip")
                nc.vector.reciprocal(recip, sumt_ps[:, 0:1])
                nc.vector.tensor_scalar_mul(o_sb, o_acc[qg][qs], recip)
                nc.sync.dma_start(
                    out[qoff + qs * P:qoff + (qs + 1) * P, :], o_sb)


def _get_nc():
    if "nc" not in _built:
        _built["nc"] = _build()
    return _built["nc"]


def _host_inputs(x, W):
    """Build the 8 per-core input maps from the full inputs."""
    x = np.asarray(x)
    W = np.asarray(W)
    w_bf = W.astype(BF16)

    in_maps = []
    for core in range(NCORES):
        b, r = divmod(core, 4)
        rows_a = slice(512 * r, 512 * r + 512)
        rows_b = slice(512 * (7 - r), 512 * (7 - r) + 512)
        xq = np.concatenate([x[b, rows_a], x[b, rows_b]], axis=0)  # [1024, D]
        xkv = x[b, 1024 * r:1024 * (r + 1)]                        # [1024, D]
        in_maps.append({
            "xq_t": np.ascontiguousarray(xq.T).astype(BF16),
            "xkv_t": np.ascontiguousarray(xkv.T).astype(BF16),
            "w": w_bf,
            "maskt": _masks_for_rank(r),
        })
    return in_maps


_mask_cache = {}


def _masks_for_rank(r):
    if r in _mask_cache:
        return _mask_cache[r]
    qpos = np.empty(NQ, dtype=np.int64)
    qpos[:512] = 512 * r + np.arange(512)
    qpos[512:] = 512 * (7 - r) + np.arange(512)
    m = np.zeros((NSTEP, P, 2, QG), dtype=BF16)
    step = 0
    for q in range(4):
        for qg in range(NQG):
            qp = qpos[qg * QG:(qg + 1) * QG]
            for rr in RRS[qg]:
                for kt in range(2):
                    kpos = 1024 * rr + QK * q + kt * P + np.arange(P)
                    m[step, :, kt, :] = (
                        kpos[:, None] <= qp[None, :]).astype(BF16)
                step += 1
    assert step == NSTEP
    _mask_cache[r] = m
    return m


def _gather(results):
    out = np.empty((B, S, D), dtype=np.float32)
    for core in range(NCORES):
        b, r = divmod(core, 4)
        co = results[core]["out"]
        out[b, 512 * r:512 * r + 512] = co[:512]
        out[b, 512 * (7 - r):512 * (7 - r) + 512] = co[512:]
    return out


def kernel(x, W):
    global LAST_EXEC_NS
    from concourse import bass_utils

    nc = _get_nc()
    in_maps = _host_inputs(x, W)
    trace = os.environ.get("BASS_KERNEL_TRACE", "0") == "1"
    if trace:
        try:
            import antenv.axon_hooks as ah
            ah.install_default_hook()
        except Exception:
            pass
    res = bass_utils.run_bass_kernel_spmd(
        nc, in_maps, core_ids=list(range(NCORES)), trace=trace,
        tmpdir=os.environ.get("BASS_KERNEL_TRACE_DIR") or None,
    )
    LAST_EXEC_NS = res.exec_time_ns
    return _gather(res.results)


# revision 27
# speedup vs baseline: 1.1647x; 1.0485x over previous
"""Fused causal attention head (QKV proj + causal softmax attention) on 8 trn2 cores.

Sharding: core = 4*b + r (b = batch of 2, r = rank in a 4-core group).
  - Queries: core handles row chunks [512r, 512r+512) and [512(7-r), 512(8-r))
    of its batch (pairing r with 7-r balances causal attention work).
  - K/V: core projects keys [1024r, 1024(r+1)); shards are exchanged with 8
    pipelined AllGathers (K and V per rank-local key *quarter*) inside each
    4-core batch group. Attention consumes quarters as they arrive
    (quarter-major loop), hiding the ~65 GB/s interconnect behind compute.
Attention is computed in a transposed-scores layout (keys on PSUM partitions):
  S^T = K^T-chunk^T-matmul-Q^T, P^T = exp(S^T/32) * causal01mask,
  partial O/rowsum accumulate in PSUM per (quarter, query-group), then fold
  into SBUF fp32 accumulators; final O /= rowsum.
  No max-subtraction: scores are ~N(0,1) so exp cannot overflow fp32.
"""

import os
import sys

sys.path.insert(0, "/opt/trn_rl_repo")

import numpy as np
import ml_dtypes

B, S, D = 2, 4096, 1024
NCORES = 8
P = 128
NQ = 1024          # queries per core
QG = 256           # query group (scores matmul free dim)
NQG = NQ // QG     # 4
KB = 512
QK = 256           # quarter-of-rank key block
BF16 = ml_dtypes.bfloat16

# ranks visible per query group (qg 0/1 = early chunk, keys < 2048)
RRS = ((0, 1), (0, 1), (0, 1, 2, 3), (0, 1, 2, 3))
NSTEP = 4 * sum(len(r) for r in RRS)  # 48 mask tiles

LAST_EXEC_NS = None
WARMUP = int(os.environ.get("KWARMUP", "12"))

_built = {}


def _build():
    import concourse.bacc as bacc
    import concourse.tile as tile
    import concourse.mybir as mybir
    from concourse.masks import make_identity

    nc = bacc.Bacc("TRN2", target_bir_lowering=False, debug=False,
                   num_devices=NCORES)
    dt = mybir.dt

    xq_t = nc.dram_tensor("xq_t", [D, NQ], dt.bfloat16, kind="ExternalInput").ap()
    xkv_t = nc.dram_tensor("xkv_t", [D, 1024], dt.bfloat16, kind="ExternalInput").ap()
    w = nc.dram_tensor("w", [D, 3 * D], dt.bfloat16, kind="ExternalInput").ap()
    maskt = nc.dram_tensor("maskt", [NSTEP, P, 2, QG], dt.bfloat16,
                           kind="ExternalInput").ap()
    out = nc.dram_tensor("out", [NQ, D], dt.float32, kind="ExternalOutput").ap()

    DC = D // P  # 8 contraction chunks
    RG = [[0, 1, 2, 3], [4, 5, 6, 7]]

    with tile.TileContext(nc, num_cores=NCORES) as tc:
        with (
            tc.tile_pool(name="persist", bufs=1) as persist,
            tc.tile_pool(name="dram", bufs=1, space="DRAM") as dram,
        ):
            qt_sb = persist.tile([P, DC, NQ], dt.bfloat16)
            ones_sb = persist.tile([P, P], dt.bfloat16)
            nc.vector.memset(ones_sb, 1.0)
            ident = persist.tile([P, P], dt.float32)
            make_identity(nc, ident)

            agin_k = [dram.tile([1024, QK], dt.bfloat16, name=f"agin_k{q}")
                      for q in range(4)]
            agout_k = [dram.tile([4096, QK], dt.bfloat16, name=f"agout_k{q}")
                       for q in range(4)]
            agin_v = [dram.tile([QK, 1024], dt.bfloat16, name=f"agin_v{q}")
                      for q in range(4)]
            agout_v = [dram.tile([4 * QK, 1024], dt.bfloat16,
                                 name=f"agout_v{q}") for q in range(4)]

            # ---- Phase 1: projections + 8 pipelined AllGathers ----
            with (
                tc.tile_pool(name="projbuf", bufs=1) as projbuf,
                tc.tile_pool(name="projtmp", bufs=4) as projtmp,
                tc.tile_pool(name="projps", bufs=4, space="PSUM") as projps,
            ):
                # PE warmup while input DMAs stream
                if WARMUP:
                    wu = projbuf.tile([P, KB], dt.bfloat16)
                    nc.vector.memset(wu, 0.0)
                    wu_ps = projps.tile([P, KB], dt.float32, tag="pps",
                                        name="wu_ps")
                    for i in range(WARMUP):
                        nc.tensor.matmul(wu_ps, lhsT=wu[:, :P], rhs=wu,
                                         start=True, stop=True)

                w_sb = projbuf.tile([P, DC, 3 * D], dt.bfloat16)
                xkv_sb = projbuf.tile([P, DC, 1024], dt.bfloat16)
                xq_sb = projbuf.tile([P, DC, NQ], dt.bfloat16)
                nc.sync.dma_start(xkv_sb, xkv_t.rearrange("(c p) n -> p c n", p=P))
                w_r = w.rearrange("(c p) n -> p c n", p=P)
                nc.sync.dma_start(w_sb[:, :, D:2 * D], w_r[:, :, D:2 * D])
                nc.sync.dma_start(w_sb[:, :, 2 * D:3 * D], w_r[:, :, 2 * D:3 * D])
                nc.sync.dma_start(w_sb[:, :, 0:D], w_r[:, :, 0:D])
                nc.sync.dma_start(xq_sb, xq_t.rearrange("(c p) n -> p c n", p=P))

                def proj_k_quarter(q):
                    agin_k_r = agin_k[q].rearrange("(m p) k -> m p k", p=P)
                    for m in range(DC):
                        kt_ps = projps.tile([P, QK], dt.float32, tag="ppsk",
                                            name="kt_ps")
                        for c in range(DC):
                            nc.tensor.matmul(
                                kt_ps,
                                lhsT=w_sb[:, c, D + m * P:D + (m + 1) * P],
                                rhs=xkv_sb[:, c, q * QK:(q + 1) * QK],
                                start=(c == 0), stop=(c == DC - 1),
                            )
                        kt_bf = projtmp.tile([P, QK], dt.bfloat16, tag="pck")
                        nc.vector.tensor_copy(kt_bf, kt_ps)
                        nc.sync.dma_start(agin_k_r[m], kt_bf)
                    nc.gpsimd.collective_compute(
                        "AllGather", mybir.AluOpType.bypass, replica_groups=RG,
                        ins=[agin_k[q].opt()], outs=[agout_k[q].opt()])

                def proj_v_quarter(q):
                    agin_v_r = agin_v[q].rearrange("(m p) d -> m p d", p=P)
                    for m in range(2):
                        for nh in range(2):
                            v_ps = projps.tile([P, KB], dt.float32, tag="pps",
                                               name="v_ps")
                            for c in range(DC):
                                nc.tensor.matmul(
                                    v_ps,
                                    lhsT=xkv_sb[:, c,
                                                q * QK + m * P:
                                                q * QK + (m + 1) * P],
                                    rhs=w_sb[:, c,
                                             2 * D + nh * KB:
                                             2 * D + (nh + 1) * KB],
                                    start=(c == 0), stop=(c == DC - 1),
                                )
                            v_bf = projtmp.tile([P, KB], dt.bfloat16,
                                                tag="pcopy")
                            nc.vector.tensor_copy(v_bf, v_ps)
                            nc.sync.dma_start(
                                agin_v_r[m][:, nh * KB:(nh + 1) * KB], v_bf)
                    nc.gpsimd.collective_compute(
                        "AllGather", mybir.AluOpType.bypass, replica_groups=RG,
                        ins=[agin_v[q].opt()], outs=[agout_v[q].opt()])

                # AG wire order: K0 K1 V0 V1 K2 K3 V2 V3 — scores run ahead
                # on K arrivals while PV drains behind V arrivals.
                proj_k_quarter(0)
                proj_k_quarter(1)
                proj_v_quarter(0)
                proj_v_quarter(1)
                proj_k_quarter(2)
                proj_k_quarter(3)
                proj_v_quarter(2)
                proj_v_quarter(3)

                # Q^T: [dout, q]  (overlaps the AllGathers)
                for m in range(DC):
                    for nh in range(2):
                        q_ps = projps.tile([P, KB], dt.float32, tag="pps",
                                           name="q_ps")
                        for c in range(DC):
                            nc.tensor.matmul(
                                q_ps,
                                lhsT=w_sb[:, c, m * P:(m + 1) * P],
                                rhs=xq_sb[:, c, nh * KB:(nh + 1) * KB],
                                start=(c == 0), stop=(c == DC - 1),
                            )
                        nc.vector.tensor_copy(
                            qt_sb[:, m, nh * KB:(nh + 1) * KB], q_ps)

            # ---- Phase 2: attention, quarter-major ----
            _phase2(nc, tc, mybir, qt_sb, ones_sb, ident,
                    agout_k, agout_v, maskt, out)

    nc.compile()
    return nc


def _phase2(nc, tc, mybir, qt_sb, ones_sb, ident, agout_k, agout_v, maskt,
            out):
    dt = mybir.dt
    DC = D // P
    with (
        tc.tile_pool(name="acc", bufs=1) as accpool,
        tc.tile_pool(name="kvq", bufs=2) as kvqpool,
        tc.tile_pool(name="mask", bufs=4) as maskpool,
        tc.tile_pool(name="pt", bufs=3) as ptpool,
        tc.tile_pool(name="norm", bufs=2) as normpool,
        tc.tile_pool(name="osb", bufs=2) as osbpool,
        tc.tile_pool(name="ops", bufs=5, space="PSUM") as opspool,
        tc.tile_pool(name="stps", bufs=2, space="PSUM") as stpspool,
        tc.tile_pool(name="sumps", bufs=1, space="PSUM") as sumpspool,
    ):
        o_acc = [[accpool.tile([P, D], dt.float32, name=f"oacc{qg}_{qs}")
                  for qs in range(2)] for qg in range(NQG)]
        sum_acc = [accpool.tile([P, QG], dt.float32, name=f"sacc{qg}")
                   for qg in range(NQG)]

        ktq = {}       # (q, rr) -> K^T tile
        vq = {}        # (q, rr) -> V tile
        pt_tiles = {}  # (q, qg, rr, kt) -> P^T tile
        state = {"step": 0}

        def load_k(q):
            for rr in range(4):
                kt_t = kvqpool.tile([P, DC, QK], dt.bfloat16, tag=f"ktq{rr}",
                                    name=f"ktq{q}_{rr}")
                nc.sync.dma_start(
                    kt_t,
                    agout_k[q][1024 * rr:1024 * (rr + 1)]
                    .rearrange("(c p) k -> p c k", p=P))
                ktq[(q, rr)] = kt_t

        def load_v(q):
            for rr in range(4):
                v_t = kvqpool.tile([P, 2, 1024], dt.bfloat16, tag=f"vq{rr}",
                                   name=f"vq{q}_{rr}")
                nc.sync.dma_start(
                    v_t,
                    agout_v[q][QK * rr:QK * (rr + 1)]
                    .rearrange("(c p) d -> p c d", p=P))
                vq[(q, rr)] = v_t

        def pass_scores(q):
            for qg in range(NQG):
                qoff = qg * QG
                for rr in RRS[qg]:
                    mask_sb = maskpool.tile([P, 2, QG], dt.bfloat16,
                                            tag="mask")
                    nc.sync.dma_start(mask_sb, maskt[state["step"]])
                    for kt in range(2):
                        st_ps = stpspool.tile([P, QG], dt.float32, tag="st")
                        for c in range(DC):
                            nc.tensor.matmul(
                                st_ps,
                                lhsT=ktq[(q, rr)][:, c, kt * P:(kt + 1) * P],
                                rhs=qt_sb[:, c, qoff:qoff + QG],
                                start=(c == 0), stop=(c == DC - 1),
                            )
                        pt_sb = ptpool.tile([P, QG], dt.bfloat16, tag="pt",
                                            bufs=52,
                                            name=f"pt{q}_{qg}_{rr}_{kt}")
                        nc.scalar.activation(
                            out=pt_sb, in_=st_ps,
                            func=mybir.ActivationFunctionType.Exp,
                            scale=float(1.0 / np.sqrt(D)),
                        )
                        nc.vector.tensor_mul(pt_sb, pt_sb, mask_sb[:, kt, :])
                        pt_tiles[(q, qg, rr, kt)] = pt_sb
                    state["step"] += 1

        def pass_pv(q):
            for qg in range(NQG):
                rrs = RRS[qg]
                # four 1-bank partial-O tiles (qs, dn) with 5 slots so the
                # next (quarter, qg) can start accumulating while folds drain
                o_ps = [opspool.tile([P, KB], dt.float32, tag="opart", bufs=5,
                                     name=f"o_{q}_{qg}_{i}")
                        for i in range(4)]
                sum_ps = sumpspool.tile([P, QG], dt.float32, tag="sum_ps")
                for rr in rrs:
                    for kt in range(2):
                        pt_sb = pt_tiles.pop((q, qg, rr, kt))
                        mm_start = rr == rrs[0] and kt == 0
                        mm_stop = rr == rrs[-1] and kt == 1
                        for qs in range(2):
                            for dn in range(2):
                                nc.tensor.matmul(
                                    o_ps[qs * 2 + dn],
                                    lhsT=pt_sb[:, qs * P:(qs + 1) * P],
                                    rhs=vq[(q, rr)][:, kt,
                                                    dn * KB:(dn + 1) * KB],
                                    start=mm_start, stop=mm_stop,
                                )
                        nc.tensor.matmul(
                            sum_ps, lhsT=ones_sb, rhs=pt_sb,
                            start=mm_start, stop=mm_stop,
                        )
                # fold partials into SBUF accumulators
                for qs in range(2):
                    for dn in range(2):
                        dst = o_acc[qg][qs][:, dn * KB:(dn + 1) * KB]
                        if q == 0:
                            nc.vector.tensor_copy(dst, o_ps[qs * 2 + dn])
                        else:
                            nc.vector.tensor_add(dst, dst, o_ps[qs * 2 + dn])
                if q == 0:
                    nc.vector.tensor_copy(sum_acc[qg], sum_ps)
                else:
                    nc.vector.tensor_add(sum_acc[qg], sum_acc[qg], sum_ps)

        # emission order matches the AG wire order K0 K1 V0 V1 K2 K3 V2 V3
        load_k(0); pass_scores(0)
        load_k(1); pass_scores(1)
        load_v(0); pass_pv(0)
        load_v(1); pass_pv(1)
        load_k(2); pass_scores(2)
        load_k(3); pass_scores(3)
        load_v(2); pass_pv(2)
        load_v(3); pass_pv(3)

        assert state["step"] == NSTEP


        # ---- normalize: O /= rowsum ----
        for qg in range(NQG):
            qoff = qg * QG
            for qs in range(2):
                o_sb = osbpool.tile([P, D], dt.float32, tag="o_sb")
                sumt_ps = stpspool.tile([P, P], dt.float32, tag="st")
                nc.tensor.transpose(
                    sumt_ps, sum_acc[qg][:, qs * P:(qs + 1) * P], ident)
                recip = normpool.tile([P, 1], dt.float32, tag="recip")
                nc.vector.reciprocal(recip, sumt_ps[:, 0:1])
                nc.vector.tensor_scalar_mul(o_sb, o_acc[qg][qs], recip)
                nc.sync.dma_start(
                    out[qoff + qs * P:qoff + (qs + 1) * P, :], o_sb)


def _get_nc():
    if "nc" not in _built:
        _built["nc"] = _build()
    return _built["nc"]


def _host_inputs(x, W):
    """Build the 8 per-core input maps from the full inputs."""
    x = np.asarray(x)
    W = np.asarray(W)
    w_bf = W.astype(BF16)

    in_maps = []
    for core in range(NCORES):
        b, r = divmod(core, 4)
        rows_a = slice(512 * r, 512 * r + 512)
        rows_b = slice(512 * (7 - r), 512 * (7 - r) + 512)
        xq = np.concatenate([x[b, rows_a], x[b, rows_b]], axis=0)  # [1024, D]
        xkv = x[b, 1024 * r:1024 * (r + 1)]                        # [1024, D]
        in_maps.append({
            "xq_t": np.ascontiguousarray(xq.T).astype(BF16),
            "xkv_t": np.ascontiguousarray(xkv.T).astype(BF16),
            "w": w_bf,
            "maskt": _masks_for_rank(r),
        })
    return in_maps


_mask_cache = {}


def _masks_for_rank(r):
    if r in _mask_cache:
        return _mask_cache[r]
    qpos = np.empty(NQ, dtype=np.int64)
    qpos[:512] = 512 * r + np.arange(512)
    qpos[512:] = 512 * (7 - r) + np.arange(512)
    m = np.zeros((NSTEP, P, 2, QG), dtype=BF16)
    step = 0
    for q in range(4):
        for qg in range(NQG):
            qp = qpos[qg * QG:(qg + 1) * QG]
            for rr in RRS[qg]:
                for kt in range(2):
                    kpos = 1024 * rr + QK * q + kt * P + np.arange(P)
                    m[step, :, kt, :] = (
                        kpos[:, None] <= qp[None, :]).astype(BF16)
                step += 1
    assert step == NSTEP
    _mask_cache[r] = m
    return m


def _gather(results):
    out = np.empty((B, S, D), dtype=np.float32)
    for core in range(NCORES):
        b, r = divmod(core, 4)
        co = results[core]["out"]
        out[b, 512 * r:512 * r + 512] = co[:512]
        out[b, 512 * (7 - r):512 * (7 - r) + 512] = co[512:]
    return out


def kernel(x, W):
    global LAST_EXEC_NS
    from concourse import bass_utils

    nc = _get_nc()
    in_maps = _host_inputs(x, W)
    trace = os.environ.get("BASS_KERNEL_TRACE", "0") == "1"
    if trace:
        try:
            import antenv.axon_hooks as ah
            ah.install_default_hook()
        except Exception:
            pass
    res = bass_utils.run_bass_kernel_spmd(
        nc, in_maps, core_ids=list(range(NCORES)), trace=trace,
        tmpdir=os.environ.get("BASS_KERNEL_TRACE_DIR") or None,
    )
    LAST_EXEC_NS = res.exec_time_ns
    return _gather(res.results)
